# revision 41
# baseline (speedup 1.0000x reference)
"""BiDAF attention kernel for 8 Trainium2 NeuronCores (data-parallel over batch).

Contract: kernel(**inputs) takes the FULL unsharded inputs (as produced by the
reference setup_inputs) and returns the FULL [16, 1024, 2048] fp32 output.

Math (per batch b):
    s[i,j]  = c[i].c_w + q[j].q_w + sum_h c[i,h]*cqw[h]*q[j,h] + bias
    s1      = softmax_j(masked(s, q_mask));  s2 = softmax_i(masked(s, c_mask))
    a       = s1 @ q ; bb = s1 @ s2^T @ c
    out     = concat(c, a, c*a, c*bb)

Device mapping (per core: 2 batches):
  - Host folds cq_weight and c_weight into the q side:  qw'[j,h] = q*cqw + c_w
    so one PE matmul chain gives sT[j,i] = sim_cq[i,j] + sim_c[i].
  - sim_q + bias + q_mask fold into the Exp activation's per-partition bias.
  - c_mask (when non-trivial) is added via a K=1 rank-1 matmul.
  - Softmax without max-subtraction (values bounded, fp32-safe): one exp(sT)
    serves both softmaxes; normalizations are per-partition scales applied to
    the downstream matmul outputs.
  - t = s2T@c via PE-transposed exp(sT); a/b via e as stationary operand.
  - The c block of the output is assembled on the host (pure copy of an
    input); the device emits only the computed a | c*a | c*b blocks.

Precision modes (BIDAF_DTYPE): "mixed" (default; similarity chain fp32,
post-softmax matmuls float32r), "fp32" (all exact), "fp32r" (all reduced).
"""

import os
import sys
from contextlib import ExitStack

import numpy as np

for _p in ("/opt/trn_rl_repo", "/root/.axon_site/_ro/trn_rl_repo"):
    if os.path.isdir(_p) and _p not in sys.path:
        sys.path.append(_p)

B, CL, QL, H = 16, 1024, 128, 512
N_CORES = 8
BPC = B // N_CORES  # batches per core
NEG = np.float32(-1e30)

DTYPE_MODE = os.environ.get("BIDAF_DTYPE", "fp32r")

_build_cache = {}


def _build(mask_trivial: bool, mode: str):
    key = (mask_trivial, mode)
    if key in _build_cache:
        return _build_cache[key]

    import concourse.bass as bass
    import concourse.tile as tile
    from concourse import bacc, mybir

    F32 = mybir.dt.float32
    F32R = mybir.dt.float32r
    SIM_DT = F32R if mode == "fp32r" else F32  # similarity-chain matmul dtype
    DOWN_DT = F32 if mode == "fp32" else F32R  # post-softmax matmul dtype
    AF = mybir.ActivationFunctionType
    PSUM = bass.MemorySpace.PSUM

    nc = bacc.Bacc("TRN2", target_bir_lowering=False, debug=False)

    c_d = nc.dram_tensor("c", [BPC, CL, H], F32, kind="ExternalInput")
    # qpack[:, 0:512] = qw'.T tiles, [:, 512] = qbias, [:, 513:1025] = q
    qpack_d = nc.dram_tensor("qpack", [BPC, 128, 1025], F32, kind="ExternalInput")
    ident_d = nc.dram_tensor("ident", [128, 128], F32, kind="ExternalInput")
    if not mask_trivial:
        cmask_d = nc.dram_tensor("cmaskb", [BPC, 1, CL], F32, kind="ExternalInput")
        onesr_d = nc.dram_tensor("onesr", [1, QL], F32, kind="ExternalInput")
    onesc_d = nc.dram_tensor("onesc", [QL, 1], F32, kind="ExternalInput")
    aca_d = nc.dram_tensor("out_aca", [BPC, CL, 2 * H], F32, kind="ExternalOutput")
    cb_d = nc.dram_tensor("out_cb", [BPC, CL, H], F32, kind="ExternalOutput")

    KT = H // 128  # 4 k-tiles over the hidden dim
    IT = CL // 128  # 8 i-tiles over the context dim

    with tile.TileContext(nc) as tc, ExitStack() as ctx:
        const = ctx.enter_context(tc.tile_pool(name="const", bufs=1))
        sbp = ctx.enter_context(tc.tile_pool(name="sbp", bufs=2))
        outp = ctx.enter_context(tc.tile_pool(name="outp", bufs=4))
        ps_acc = ctx.enter_context(tc.tile_pool(name="ps_acc", bufs=2, space=PSUM))
        ps_tr = ctx.enter_context(tc.tile_pool(name="ps_tr", bufs=2, space=PSUM))
        ps_ab = ctx.enter_context(tc.tile_pool(name="ps_ab", bufs=3, space=PSUM))
        ps_cs = ctx.enter_context(tc.tile_pool(name="ps_cs", bufs=1, space=PSUM))

        ident = const.tile([128, 128], F32, tag="ident")
        nc.sync.dma_start(ident[:], ident_d.ap())
        if DOWN_DT != F32:
            ident_e = const.tile([128, 128], DOWN_DT, tag="ident_e")
            nc.vector.tensor_copy(ident_e[:], ident[:])
        else:
            ident_e = ident
        if not mask_trivial:
            cmask_f = const.tile([1, BPC * CL], F32, tag="cmask_f")
            nc.sync.dma_start(cmask_f[:], cmask_d.ap().rearrange("b one i -> one (b i)"))
            onesr_f = const.tile([1, QL], F32, tag="onesr_f")
            nc.sync.dma_start(onesr_f[:], onesr_d.ap())
            if SIM_DT != F32:
                cmask_all = const.tile([1, BPC * CL], SIM_DT, tag="cmask")
                nc.vector.tensor_copy(cmask_all[:], cmask_f[:])
                onesr = const.tile([1, QL], SIM_DT, tag="onesr")
                nc.vector.tensor_copy(onesr[:], onesr_f[:])
            else:
                cmask_all, onesr = cmask_f, onesr_f

        onesc_f = const.tile([QL, 1], F32, tag="onesc_f")
        nc.sync.dma_start(onesc_f[:], onesc_d.ap())

        # ---- phase A: emit ALL loads (both batches). c loads cast to the
        # matmul dtype in-flight (SWDGE); everything else on the SP HWDGE
        # queue. CDT values feed transposes/traw/cr1; the output's exact c
        # block is assembled host-side, so rounding c here only perturbs the
        # already-approximate a/ca/cb products.
        CDT = DOWN_DT if mode == "fp32r" else F32
        LD = []
        for bi in range(BPC):
            ch = []
            for qt in range(4):
                cht = sbp.tile([128, 2, H], CDT, tag=f"cq{qt}")
                src = c_d.ap()[bi, qt * 256 : (qt + 1) * 256, :].rearrange(
                    "(t p) h -> p t h", p=128
                )
                if CDT != F32:
                    nc.gpsimd.dma_start(cht[:], src)
                else:
                    nc.sync.dma_start(cht[:], src)
                ch.append(cht)
            qpk = sbp.tile([128, 1025], F32, tag="qpk")
            nc.sync.dma_start(qpk[:], qpack_d.ap()[bi])
            LD.append((ch, qpk))

        # ---- PE clock warmup + ACT exp-table preload in the preamble window:
        # the HAM clock gate needs ~3.4us of sustained PE activity to lift the
        # PE from 1.2 to 2.4 GHz, and the first Exp pays a ~2.7us table load.
        # Both run on dummy data before the first input DMA lands.
        BF16 = mybir.dt.bfloat16
        warmf = const.tile([128, 1], F32, tag="warmf")
        nc.gpsimd.memset(warmf[:], 0.0)
        nc.scalar.activation(warmf[:, 0:1], warmf[:, 0:1], AF.Exp)
        warmL = const.tile([128, 1], BF16, tag="warmL")
        warmC = const.tile([128, 512], BF16, tag="warmC")
        nc.gpsimd.memset(warmL[:], 0.0)
        nc.gpsimd.memset(warmC[:], 0.0)
        pw = ps_tr.tile([128, 512], F32, tag="tr")
        for _ in range(16):
            nc.tensor.matmul(pw[:1, :], warmL[:], warmC[:], start=True, stop=True)

        # ---- phase B1: per-batch similarity front-end + a|c*a stores ----
        ST = []
        for bi in range(BPC):
            ch, qpk = LD[bi]
            c_sb = [ch[it // 2][:, it % 2, :] for it in range(IT)]
            qbias_sb = qpk[:, 512:513]
            if SIM_DT != F32:
                qwT_sb = sbp.tile([128, KT, QL], SIM_DT, tag="qwT_sb")
                nc.vector.tensor_copy(
                    qwT_sb[:], qpk[:, 0:512].rearrange("p (t j) -> p t j", t=KT)
                )
            else:
                qwT_sb = qpk[:, 0:512].rearrange("p (t j) -> p t j", t=KT)
            if DOWN_DT != F32:
                q_sb = sbp.tile([QL, H], DOWN_DT, tag="q_sb")
                nc.vector.tensor_copy(q_sb[:], qpk[:, 513:1025])
            else:
                q_sb = qpk[:, 513:1025]

            # ---- per CL-half front end: cT transposes, sT, exp, colsums,
            #      then the a | c*a column stores (independent of the b path)
            ehalf = []
            r1h = []
            cr1s = []
            rs2 = sbp.tile([QL, 2], F32, tag="rs2")
            for nh in range(2):
                cth = sbp.tile([128, KT, 512], SIM_DT, tag=f"ct{nh}")
                for j in range(4):
                    it = 4 * nh + j
                    ptr = ps_tr.tile([128, 512], CDT, tag="tr")
                    for k in range(KT):
                        nc.tensor.transpose(
                            ptr[:, k * 128 : (k + 1) * 128],
                            c_sb[it][:, k * 128 : (k + 1) * 128],
                            ident_e[:] if CDT != F32 else ident[:],
                        )
                    src = ptr[:].rearrange("p (k i) -> p k i", k=KT)
                    if it % 2 == 0:
                        nc.vector.tensor_copy(cth[:, :, j * 128 : (j + 1) * 128], src)
                    else:
                        nc.scalar.copy(cth[:, :, j * 128 : (j + 1) * 128], src)

                spt = ps_acc.tile([QL, 512], F32, tag="acc")
                for k in range(KT):
                    nc.tensor.matmul(
                        spt[:],
                        qwT_sb[:, k, :],
                        cth[:, k, :],
                        start=(k == 0),
                        stop=(k == KT - 1 and mask_trivial),
                    )
                if not mask_trivial:
                    nc.tensor.matmul(
                        spt[:],
                        onesr[:],
                        cmask_all[:, bi * CL + nh * 512 : bi * CL + (nh + 1) * 512],
                        start=False,
                        stop=True,
                    )

                eh = sbp.tile([QL, 512], DOWN_DT, tag=f"e{nh}")
                nc.scalar.activation(
                    eh[:],
                    spt[:],
                    AF.Exp,
                    bias=qbias_sb[:],
                    scale=1.0,
                    accum_out=rs2[:, nh : nh + 1],
                )
                ehalf.append(eh)

                pcs = ps_cs.tile([128, 4], F32, tag="cs")
                for j in range(4):
                    nc.tensor.matmul(
                        pcs[:, j : j + 1],
                        eh[:, j * 128 : (j + 1) * 128].bitcast(F32),
                        onesc_f[:],
                        start=True,
                        stop=True,
                    )
                r1n = sbp.tile([128, 4], F32, tag=f"r1{nh}")
                nc.vector.reciprocal(r1n[:], pcs[:])
                r1h.append(r1n)

                # a | c*a for this half's i-tiles (cr1 = r1*c folds the
                # softmax normalization into the c-products)
                for j in range(4):
                    it = 4 * nh + j
                    esl = eh[:, j * 128 : (j + 1) * 128]
                    pa = ps_ab.tile([128, H], F32, tag="ab")
                    nc.tensor.matmul(pa[:], esl, q_sb[:], start=True, stop=True)
                    cr1 = sbp.tile([128, H], F32, tag=f"cr1_{it}")
                    nc.vector.tensor_scalar_mul(cr1[:], c_sb[it], r1n[:, j : j + 1])
                    cr1s.append(cr1)
                    aca_sb = outp.tile([128, 2 * H], F32, tag="aca")
                    nc.scalar.mul(aca_sb[:, 0:H], pa[:], r1n[:, j : j + 1])
                    nc.vector.tensor_mul(aca_sb[:, H : 2 * H], cr1[:], pa[:])
                    rows = aca_d.ap()[bi, it * 128 : (it + 1) * 128]
                    nc.sync.dma_start(rows[:], aca_sb[:])

            ST.append((c_sb, ehalf, r1h, cr1s, rs2))

        # ---- phase B2: per-batch b path: r2, eN, traw, t, c*b stores ----
        for bi in range(BPC):
            c_sb, ehalf, r1h, cr1s, rs2 = ST[bi]
            rsum = sbp.tile([QL, 1], F32, tag="rsum")
            nc.vector.tensor_reduce(rsum[:], rs2[:], mybir.AxisListType.X, mybir.AluOpType.add)
            r2 = sbp.tile([QL, 1], F32, tag="r2")
            nc.vector.reciprocal(r2[:], rsum[:])

            eN = []
            for half in range(2):
                pe = ps_tr.tile([128, 512], DOWN_DT, tag="tr")
                for j in range(4):
                    nc.tensor.transpose(
                        pe[:, j * 128 : (j + 1) * 128],
                        ehalf[half][:, j * 128 : (j + 1) * 128],
                        ident_e[:],
                    )
                eNh = sbp.tile([128, 4, 128], CDT, tag=f"eN{half}")
                if half == 0:
                    nc.vector.tensor_copy(eNh[:], pe[:].rearrange("p (t j) -> p t j", t=4))
                else:
                    nc.scalar.copy(eNh[:], pe[:].rearrange("p (t j) -> p t j", t=4))
                eN.append(eNh)

            ptraw = ps_acc.tile([QL, H], F32, tag="acc")
            for it in range(IT):
                nc.tensor.matmul(
                    ptraw[:],
                    eN[it // 4][:, it % 4, :],
                    c_sb[it],
                    start=(it == 0),
                    stop=(it == IT - 1),
                )
            t_sb = sbp.tile([QL, H], DOWN_DT, tag="t")
            nc.scalar.mul(t_sb[:], ptraw[:], r2[:])

            for it in range(IT):
                esl = ehalf[it // 4][:, (it % 4) * 128 : (it % 4 + 1) * 128]
                pb = ps_ab.tile([128, H], F32, tag="ab")
                nc.tensor.matmul(pb[:], esl, t_sb[:], start=True, stop=True)
                cb_sb = outp.tile([128, H], F32, tag="cb")
                nc.vector.tensor_mul(cb_sb[:], cr1s[it][:], pb[:])
                rows = cb_d.ap()[bi, it * 128 : (it + 1) * 128]
                nc.sync.dma_start(rows[:], cb_sb[:])

    nc.compile()
    _build_cache[key] = nc
    return nc


def _install_profshim():
    """Optional NTFF profiling support (BIDAF_PROFILE=1); self-contained."""
    import contextlib
    import ctypes
    import types

    if "antenv.axon_hooks" in sys.modules:
        return
    so_path = "/opt/axon/libaxon_pjrt.so"
    try:
        lib = ctypes.CDLL(so_path)
    except OSError:
        return
    if not hasattr(lib, "axon_start_nrt_profile"):
        return
    lib.axon_start_nrt_profile.argtypes = [ctypes.POINTER(ctypes.c_int64), ctypes.c_size_t]
    lib.axon_start_nrt_profile.restype = ctypes.c_int64
    lib.axon_stop_nrt_profile.argtypes = [ctypes.c_char_p]
    lib.axon_stop_nrt_profile.restype = ctypes.c_int64

    @contextlib.contextmanager
    def _hook(output_dir, device_ids):
        import jax

        jax.devices()
        if device_ids:
            ids = (ctypes.c_int64 * len(device_ids))(*device_ids)
            rc = lib.axon_start_nrt_profile(ids, len(device_ids))
        else:
            rc = lib.axon_start_nrt_profile(None, 0)
        if rc != 0:
            raise RuntimeError(f"axon_start_nrt_profile rc={rc}")
        try:
            yield
        finally:
            n = lib.axon_stop_nrt_profile(str(output_dir).encode())
            print(f"profile: {n} file(s) written to {output_dir}")

    mod = types.ModuleType("antenv.axon_hooks")
    mod.get_axon_ntff_profile_hook = lambda: _hook
    mod.set_axon_ntff_profile_hook = lambda h: None
    sys.modules["antenv.axon_hooks"] = mod
    import antenv

    antenv.axon_hooks = mod

    from concourse import bass_utils

    bass_utils.upload_artifacts = lambda tmpdir: f"local:{tmpdir}"


def kernel(c, q, c_mask, q_mask, c_weight, q_weight, cq_weight, bias):
    from concourse.bass_utils import run_bass_kernel_spmd

    c = np.asarray(c, dtype=np.float32)
    q = np.asarray(q, dtype=np.float32)
    c_mask = np.asarray(c_mask)
    q_mask = np.asarray(q_mask)
    c_weight = np.asarray(c_weight, dtype=np.float32)
    q_weight = np.asarray(q_weight, dtype=np.float32)
    cq_weight = np.asarray(cq_weight, dtype=np.float32)
    bias = np.asarray(bias, dtype=np.float32)

    # host-side folding (all tiny, O(B*(CL+QL)*H) at most)
    qw = q * cq_weight.reshape(1, 1, H) + c_weight.reshape(1, 1, H)  # [B, QL, H]
    sim_q = (q @ q_weight)[:, :, 0]  # [B, QL]
    amask_q = (1.0 - q_mask.astype(np.float32)) * NEG
    qbias = (sim_q + bias[0] + amask_q).astype(np.float32)  # [B, QL]
    amask_c = ((1.0 - c_mask.astype(np.float32)) * NEG).reshape(B, 1, CL)
    mask_trivial = bool((amask_c == 0).all())
    KT = H // 128
    qpack = np.empty((B, 128, 1025), dtype=np.float32)
    qpack[:, :, 0:512] = (
        qw.reshape(B, QL, KT, 128).transpose(0, 3, 2, 1).reshape(B, 128, KT * QL)
    )
    qpack[:, :, 512] = qbias
    qpack[:, :, 513:1025] = q

    profile = os.environ.get("BIDAF_PROFILE", "") == "1"
    if profile:
        _install_profshim()

    nc = _build(mask_trivial, DTYPE_MODE)

    ident = np.eye(128, dtype=np.float32)
    onesr = np.ones((1, QL), dtype=np.float32)
    in_maps = []
    for core in range(N_CORES):
        s = slice(BPC * core, BPC * (core + 1))
        m = {
            "c": np.ascontiguousarray(c[s]),
            "qpack": np.ascontiguousarray(qpack[s]),
            "ident": ident,
            "onesc": np.ones((QL, 1), dtype=np.float32),
        }
        if not mask_trivial:
            m["cmaskb"] = np.ascontiguousarray(amask_c[s])
            m["onesr"] = onesr
        in_maps.append(m)

    kw = {}
    if profile:
        kw = dict(trace=True, tmpdir=os.environ.get("BIDAF_PROFILE_DIR") or None)
    res = run_bass_kernel_spmd(nc, in_maps, list(range(N_CORES)), **kw)
    if profile and res.exec_time_ns is not None:
        print(f"[kernel] HW exec time: {res.exec_time_ns} ns")
        kernel.last_exec_time_ns = res.exec_time_ns
        kernel.last_trace = res.instructions_and_trace[1] if res.instructions_and_trace else None

    out = np.empty((B, CL, 4 * H), dtype=np.float32)
    out[:, :, 0:H] = c
    for i in range(N_CORES):
        out[BPC * i : BPC * (i + 1), :, H : 3 * H] = res.results[i]["out_aca"]
        out[BPC * i : BPC * (i + 1), :, 3 * H :] = res.results[i]["out_cb"]
    return out


kernel.last_exec_time_ns = None
kernel.last_trace = None


# revision 42
# speedup vs baseline: 1.0258x; 1.0258x over previous
"""BiDAF attention kernel for 8 Trainium2 NeuronCores (data-parallel over batch).

Contract: kernel(**inputs) takes the FULL unsharded inputs (as produced by the
reference setup_inputs) and returns the FULL [16, 1024, 2048] fp32 output.

Math (per batch b):
    s[i,j]  = c[i].c_w + q[j].q_w + sum_h c[i,h]*cqw[h]*q[j,h] + bias
    s1      = softmax_j(masked(s, q_mask));  s2 = softmax_i(masked(s, c_mask))
    a       = s1 @ q ; bb = s1 @ s2^T @ c
    out     = concat(c, a, c*a, c*bb)

Device mapping (per core: 2 batches):
  - Host folds cq_weight and c_weight into the q side:  qw'[j,h] = q*cqw + c_w
    so one PE matmul chain gives sT[j,i] = sim_cq[i,j] + sim_c[i].
  - sim_q + bias + q_mask fold into the Exp activation's per-partition bias.
  - c_mask (when non-trivial) is added via a K=1 rank-1 matmul.
  - Softmax without max-subtraction (values bounded, fp32-safe): one exp(sT)
    serves both softmaxes; normalizations are per-partition scales applied to
    the downstream matmul outputs.
  - t = s2T@c via PE-transposed exp(sT); a/b via e as stationary operand.
  - The c block of the output is assembled on the host (pure copy of an
    input); the device emits only the computed a | c*a | c*b blocks.

Precision modes (BIDAF_DTYPE): "mixed" (default; similarity chain fp32,
post-softmax matmuls float32r), "fp32" (all exact), "fp32r" (all reduced).
"""

import os
import sys
from contextlib import ExitStack

import numpy as np

for _p in ("/opt/trn_rl_repo", "/root/.axon_site/_ro/trn_rl_repo"):
    if os.path.isdir(_p) and _p not in sys.path:
        sys.path.append(_p)

B, CL, QL, H = 16, 1024, 128, 512
N_CORES = 8
BPC = B // N_CORES  # batches per core
NEG = np.float32(-1e30)

DTYPE_MODE = os.environ.get("BIDAF_DTYPE", "fp32r")

_build_cache = {}


def _build(mask_trivial: bool, mode: str):
    key = (mask_trivial, mode)
    if key in _build_cache:
        return _build_cache[key]

    import concourse.bass as bass
    import concourse.tile as tile
    from concourse import bacc, mybir

    F32 = mybir.dt.float32
    F32R = mybir.dt.float32r
    SIM_DT = F32R if mode == "fp32r" else F32  # similarity-chain matmul dtype
    DOWN_DT = F32 if mode == "fp32" else F32R  # post-softmax matmul dtype
    AF = mybir.ActivationFunctionType
    PSUM = bass.MemorySpace.PSUM

    nc = bacc.Bacc("TRN2", target_bir_lowering=False, debug=False)

    c_d = nc.dram_tensor("c", [BPC, CL, H], F32, kind="ExternalInput")
    # qpack[:, 0:512] = qw'.T tiles, [:, 512] = qbias, [:, 513:1025] = q
    qpack_d = nc.dram_tensor("qpack", [BPC, 128, 1025], F32, kind="ExternalInput")
    ident_d = nc.dram_tensor("ident", [128, 128], F32, kind="ExternalInput")
    if not mask_trivial:
        cmask_d = nc.dram_tensor("cmaskb", [BPC, 1, CL], F32, kind="ExternalInput")
        onesr_d = nc.dram_tensor("onesr", [1, QL], F32, kind="ExternalInput")
    onesc_d = nc.dram_tensor("onesc", [QL, 1], F32, kind="ExternalInput")
    aca_d = nc.dram_tensor("out_aca", [BPC, CL, 2 * H], F32, kind="ExternalOutput")
    cb_d = nc.dram_tensor("out_cb", [BPC, CL, H], F32, kind="ExternalOutput")

    KT = H // 128  # 4 k-tiles over the hidden dim
    IT = CL // 128  # 8 i-tiles over the context dim

    with tile.TileContext(nc) as tc, ExitStack() as ctx:
        const = ctx.enter_context(tc.tile_pool(name="const", bufs=1))
        sbp = ctx.enter_context(tc.tile_pool(name="sbp", bufs=2))
        outp = ctx.enter_context(tc.tile_pool(name="outp", bufs=4))
        ps_acc = ctx.enter_context(tc.tile_pool(name="ps_acc", bufs=2, space=PSUM))
        ps_tr = ctx.enter_context(tc.tile_pool(name="ps_tr", bufs=2, space=PSUM))
        ps_ab = ctx.enter_context(tc.tile_pool(name="ps_ab", bufs=3, space=PSUM))
        ps_cs = ctx.enter_context(tc.tile_pool(name="ps_cs", bufs=1, space=PSUM))

        ident = const.tile([128, 128], F32, tag="ident")
        nc.sync.dma_start(ident[:], ident_d.ap())
        if DOWN_DT != F32:
            ident_e = const.tile([128, 128], DOWN_DT, tag="ident_e")
            nc.vector.tensor_copy(ident_e[:], ident[:])
        else:
            ident_e = ident
        if not mask_trivial:
            cmask_f = const.tile([1, BPC * CL], F32, tag="cmask_f")
            nc.sync.dma_start(cmask_f[:], cmask_d.ap().rearrange("b one i -> one (b i)"))
            onesr_f = const.tile([1, QL], F32, tag="onesr_f")
            nc.sync.dma_start(onesr_f[:], onesr_d.ap())
            if SIM_DT != F32:
                cmask_all = const.tile([1, BPC * CL], SIM_DT, tag="cmask")
                nc.vector.tensor_copy(cmask_all[:], cmask_f[:])
                onesr = const.tile([1, QL], SIM_DT, tag="onesr")
                nc.vector.tensor_copy(onesr[:], onesr_f[:])
            else:
                cmask_all, onesr = cmask_f, onesr_f

        onesc_f = const.tile([QL, 1], F32, tag="onesc_f")
        nc.sync.dma_start(onesc_f[:], onesc_d.ap())

        # ---- phase A: emit ALL loads (both batches). c loads cast to the
        # matmul dtype in-flight (SWDGE); everything else on the SP HWDGE
        # queue. CDT values feed transposes/traw/cr1; the output's exact c
        # block is assembled host-side, so rounding c here only perturbs the
        # already-approximate a/ca/cb products.
        CDT = DOWN_DT if mode == "fp32r" else F32
        LD = []
        for bi in range(BPC):
            ch = []
            for qt in range(4):
                cht = sbp.tile([128, 2, H], CDT, tag=f"cq{qt}")
                src = c_d.ap()[bi, qt * 256 : (qt + 1) * 256, :].rearrange(
                    "(t p) h -> p t h", p=128
                )
                if CDT != F32:
                    nc.gpsimd.dma_start(cht[:], src)
                else:
                    nc.sync.dma_start(cht[:], src)
                ch.append(cht)
            qpk = sbp.tile([128, 1025], F32, tag="qpk")
            nc.sync.dma_start(qpk[:], qpack_d.ap()[bi])
            LD.append((ch, qpk))

        # ---- PE clock warmup + ACT exp-table preload in the preamble window:
        # the HAM clock gate needs ~3.4us of sustained PE activity to lift the
        # PE from 1.2 to 2.4 GHz, and the first Exp pays a ~2.7us table load.
        # Both run on dummy data before the first input DMA lands.
        BF16 = mybir.dt.bfloat16
        warmf = const.tile([128, 1], F32, tag="warmf")
        nc.gpsimd.memset(warmf[:], 0.0)
        nc.scalar.activation(warmf[:, 0:1], warmf[:, 0:1], AF.Exp)
        warmL = const.tile([128, 1], BF16, tag="warmL")
        warmC = const.tile([128, 512], BF16, tag="warmC")
        nc.gpsimd.memset(warmL[:], 0.0)
        nc.gpsimd.memset(warmC[:], 0.0)
        pw = ps_tr.tile([128, 512], F32, tag="tr")
        for _ in range(16):
            nc.tensor.matmul(pw[:1, :], warmL[:], warmC[:], start=True, stop=True)

        # ---- phase B: per-batch compute + stores (stores also SP HWDGE) ----
        for bi in range(BPC):
            ch, qpk = LD[bi]
            c_sb = [ch[it // 2][:, it % 2, :] for it in range(IT)]
            qbias_sb = qpk[:, 512:513]
            if SIM_DT != F32:
                qwT_sb = sbp.tile([128, KT, QL], SIM_DT, tag="qwT_sb")
                nc.vector.tensor_copy(
                    qwT_sb[:], qpk[:, 0:512].rearrange("p (t j) -> p t j", t=KT)
                )
            else:
                qwT_sb = qpk[:, 0:512].rearrange("p (t j) -> p t j", t=KT)
            if DOWN_DT != F32:
                q_sb = sbp.tile([QL, H], DOWN_DT, tag="q_sb")
                nc.vector.tensor_copy(q_sb[:], qpk[:, 513:1025])
            else:
                q_sb = qpk[:, 513:1025]

            # ---- per CL-half front end: cT transposes, sT, exp, colsums,
            #      then the a | c*a column stores (independent of the b path)
            ehalf = []
            r1h = []
            cr1s = []
            rs2 = sbp.tile([QL, 2], F32, tag="rs2")
            for nh in range(2):
                cth = sbp.tile([128, KT, 512], SIM_DT, tag=f"ct{nh}")
                for j in range(4):
                    it = 4 * nh + j
                    ptr = ps_tr.tile([128, 512], CDT, tag="tr")
                    for k in range(KT):
                        nc.tensor.transpose(
                            ptr[:, k * 128 : (k + 1) * 128],
                            c_sb[it][:, k * 128 : (k + 1) * 128],
                            ident_e[:] if CDT != F32 else ident[:],
                        )
                    src = ptr[:].rearrange("p (k i) -> p k i", k=KT)
                    if it % 2 == 0:
                        nc.vector.tensor_copy(cth[:, :, j * 128 : (j + 1) * 128], src)
                    else:
                        nc.scalar.copy(cth[:, :, j * 128 : (j + 1) * 128], src)

                spt = ps_acc.tile([QL, 512], F32, tag="acc")
                for k in range(KT):
                    nc.tensor.matmul(
                        spt[:],
                        qwT_sb[:, k, :],
                        cth[:, k, :],
                        start=(k == 0),
                        stop=(k == KT - 1 and mask_trivial),
                    )
                if not mask_trivial:
                    nc.tensor.matmul(
                        spt[:],
                        onesr[:],
                        cmask_all[:, bi * CL + nh * 512 : bi * CL + (nh + 1) * 512],
                        start=False,
                        stop=True,
                    )

                eh = sbp.tile([QL, 512], DOWN_DT, tag=f"e{nh}")
                nc.scalar.activation(
                    eh[:],
                    spt[:],
                    AF.Exp,
                    bias=qbias_sb[:],
                    scale=1.0,
                    accum_out=rs2[:, nh : nh + 1],
                )
                ehalf.append(eh)

                pcs = ps_cs.tile([128, 4], F32, tag="cs")
                for j in range(4):
                    nc.tensor.matmul(
                        pcs[:, j : j + 1],
                        eh[:, j * 128 : (j + 1) * 128].bitcast(F32),
                        onesc_f[:],
                        start=True,
                        stop=True,
                    )
                r1n = sbp.tile([128, 4], F32, tag=f"r1{nh}")
                nc.vector.reciprocal(r1n[:], pcs[:])
                r1h.append(r1n)

                # a | c*a for this half's i-tiles (cr1 = r1*c folds the
                # softmax normalization into the c-products)
                for j in range(4):
                    it = 4 * nh + j
                    esl = eh[:, j * 128 : (j + 1) * 128]
                    pa = ps_ab.tile([128, H], F32, tag="ab")
                    nc.tensor.matmul(pa[:], esl, q_sb[:], start=True, stop=True)
                    cr1 = sbp.tile([128, H], F32, tag=f"cr1_{it}")
                    nc.vector.tensor_scalar_mul(cr1[:], c_sb[it], r1n[:, j : j + 1])
                    cr1s.append(cr1)
                    aca_sb = outp.tile([128, 2 * H], F32, tag="aca")
                    nc.scalar.mul(aca_sb[:, 0:H], pa[:], r1n[:, j : j + 1])
                    nc.vector.tensor_mul(aca_sb[:, H : 2 * H], cr1[:], pa[:])
                    rows = aca_d.ap()[bi, it * 128 : (it + 1) * 128]
                    nc.sync.dma_start(rows[:], aca_sb[:])

            # ---- b path: r2, eN transposes, traw (fp32), t, then c*b stores ----
            rsum = sbp.tile([QL, 1], F32, tag="rsum")
            nc.vector.tensor_reduce(rsum[:], rs2[:], mybir.AxisListType.X, mybir.AluOpType.add)
            r2 = sbp.tile([QL, 1], F32, tag="r2")
            nc.vector.reciprocal(r2[:], rsum[:])

            eN = []
            for half in range(2):
                pe = ps_tr.tile([128, 512], DOWN_DT, tag="tr")
                for j in range(4):
                    nc.tensor.transpose(
                        pe[:, j * 128 : (j + 1) * 128],
                        ehalf[half][:, j * 128 : (j + 1) * 128],
                        ident_e[:],
                    )
                eNh = sbp.tile([128, 4, 128], CDT, tag=f"eN{half}")
                if half == 0:
                    nc.vector.tensor_copy(eNh[:], pe[:].rearrange("p (t j) -> p t j", t=4))
                else:
                    nc.scalar.copy(eNh[:], pe[:].rearrange("p (t j) -> p t j", t=4))
                eN.append(eNh)

            ptraw = ps_acc.tile([QL, H], F32, tag="acc")
            for it in range(IT):
                nc.tensor.matmul(
                    ptraw[:],
                    eN[it // 4][:, it % 4, :],
                    c_sb[it],
                    start=(it == 0),
                    stop=(it == IT - 1),
                )
            t_sb = sbp.tile([QL, H], DOWN_DT, tag="t")
            nc.scalar.mul(t_sb[:], ptraw[:], r2[:])

            for it in range(IT):
                esl = ehalf[it // 4][:, (it % 4) * 128 : (it % 4 + 1) * 128]
                pb = ps_ab.tile([128, H], F32, tag="ab")
                nc.tensor.matmul(pb[:], esl, t_sb[:], start=True, stop=True)
                cb_sb = outp.tile([128, H], F32, tag="cb")
                nc.vector.tensor_mul(cb_sb[:], cr1s[it][:], pb[:])
                rows = cb_d.ap()[bi, it * 128 : (it + 1) * 128]
                nc.sync.dma_start(rows[:], cb_sb[:])

    nc.compile()
    _build_cache[key] = nc
    return nc


def _install_profshim():
    """Optional NTFF profiling support (BIDAF_PROFILE=1); self-contained."""
    import contextlib
    import ctypes
    import types

    if "antenv.axon_hooks" in sys.modules:
        return
    so_path = "/opt/axon/libaxon_pjrt.so"
    try:
        lib = ctypes.CDLL(so_path)
    except OSError:
        return
    if not hasattr(lib, "axon_start_nrt_profile"):
        return
    lib.axon_start_nrt_profile.argtypes = [ctypes.POINTER(ctypes.c_int64), ctypes.c_size_t]
    lib.axon_start_nrt_profile.restype = ctypes.c_int64
    lib.axon_stop_nrt_profile.argtypes = [ctypes.c_char_p]
    lib.axon_stop_nrt_profile.restype = ctypes.c_int64

    @contextlib.contextmanager
    def _hook(output_dir, device_ids):
        import jax

        jax.devices()
        if device_ids:
            ids = (ctypes.c_int64 * len(device_ids))(*device_ids)
            rc = lib.axon_start_nrt_profile(ids, len(device_ids))
        else:
            rc = lib.axon_start_nrt_profile(None, 0)
        if rc != 0:
            raise RuntimeError(f"axon_start_nrt_profile rc={rc}")
        try:
            yield
        finally:
            n = lib.axon_stop_nrt_profile(str(output_dir).encode())
            print(f"profile: {n} file(s) written to {output_dir}")

    mod = types.ModuleType("antenv.axon_hooks")
    mod.get_axon_ntff_profile_hook = lambda: _hook
    mod.set_axon_ntff_profile_hook = lambda h: None
    sys.modules["antenv.axon_hooks"] = mod
    import antenv

    antenv.axon_hooks = mod

    from concourse import bass_utils

    bass_utils.upload_artifacts = lambda tmpdir: f"local:{tmpdir}"


def kernel(c, q, c_mask, q_mask, c_weight, q_weight, cq_weight, bias):
    from concourse.bass_utils import run_bass_kernel_spmd

    c = np.asarray(c, dtype=np.float32)
    q = np.asarray(q, dtype=np.float32)
    c_mask = np.asarray(c_mask)
    q_mask = np.asarray(q_mask)
    c_weight = np.asarray(c_weight, dtype=np.float32)
    q_weight = np.asarray(q_weight, dtype=np.float32)
    cq_weight = np.asarray(cq_weight, dtype=np.float32)
    bias = np.asarray(bias, dtype=np.float32)

    # host-side folding (all tiny, O(B*(CL+QL)*H) at most)
    qw = q * cq_weight.reshape(1, 1, H) + c_weight.reshape(1, 1, H)  # [B, QL, H]
    sim_q = (q @ q_weight)[:, :, 0]  # [B, QL]
    amask_q = (1.0 - q_mask.astype(np.float32)) * NEG
    qbias = (sim_q + bias[0] + amask_q).astype(np.float32)  # [B, QL]
    amask_c = ((1.0 - c_mask.astype(np.float32)) * NEG).reshape(B, 1, CL)
    mask_trivial = bool((amask_c == 0).all())
    KT = H // 128
    qpack = np.empty((B, 128, 1025), dtype=np.float32)
    qpack[:, :, 0:512] = (
        qw.reshape(B, QL, KT, 128).transpose(0, 3, 2, 1).reshape(B, 128, KT * QL)
    )
    qpack[:, :, 512] = qbias
    qpack[:, :, 513:1025] = q

    profile = os.environ.get("BIDAF_PROFILE", "") == "1"
    if profile:
        _install_profshim()

    nc = _build(mask_trivial, DTYPE_MODE)

    ident = np.eye(128, dtype=np.float32)
    onesr = np.ones((1, QL), dtype=np.float32)
    in_maps = []
    for core in range(N_CORES):
        s = slice(BPC * core, BPC * (core + 1))
        m = {
            "c": np.ascontiguousarray(c[s]),
            "qpack": np.ascontiguousarray(qpack[s]),
            "ident": ident,
            "onesc": np.ones((QL, 1), dtype=np.float32),
        }
        if not mask_trivial:
            m["cmaskb"] = np.ascontiguousarray(amask_c[s])
            m["onesr"] = onesr
        in_maps.append(m)

    kw = {}
    if profile:
        kw = dict(trace=True, tmpdir=os.environ.get("BIDAF_PROFILE_DIR") or None)
    res = run_bass_kernel_spmd(nc, in_maps, list(range(N_CORES)), **kw)
    if profile and res.exec_time_ns is not None:
        print(f"[kernel] HW exec time: {res.exec_time_ns} ns")
        kernel.last_exec_time_ns = res.exec_time_ns
        kernel.last_trace = res.instructions_and_trace[1] if res.instructions_and_trace else None

    out = np.empty((B, CL, 4 * H), dtype=np.float32)
    out[:, :, 0:H] = c
    for i in range(N_CORES):
        out[BPC * i : BPC * (i + 1), :, H : 3 * H] = res.results[i]["out_aca"]
        out[BPC * i : BPC * (i + 1), :, 3 * H :] = res.results[i]["out_cb"]
    return out


kernel.last_exec_time_ns = None
kernel.last_trace = None


# revision 43
# speedup vs baseline: 1.0474x; 1.0211x over previous
"""BiDAF attention kernel for 8 Trainium2 NeuronCores (data-parallel over batch).

Contract: kernel(**inputs) takes the FULL unsharded inputs (as produced by the
reference setup_inputs) and returns the FULL [16, 1024, 2048] fp32 output.

Math (per batch b):
    s[i,j]  = c[i].c_w + q[j].q_w + sum_h c[i,h]*cqw[h]*q[j,h] + bias
    s1      = softmax_j(masked(s, q_mask));  s2 = softmax_i(masked(s, c_mask))
    a       = s1 @ q ; bb = s1 @ s2^T @ c
    out     = concat(c, a, c*a, c*bb)

Device mapping (per core: 2 batches):
  - Host folds cq_weight and c_weight into the q side:  qw'[j,h] = q*cqw + c_w
    so one PE matmul chain gives sT[j,i] = sim_cq[i,j] + sim_c[i].
  - sim_q + bias + q_mask fold into the Exp activation's per-partition bias.
  - c_mask (when non-trivial) is added via a K=1 rank-1 matmul.
  - Softmax without max-subtraction (values bounded, fp32-safe): one exp(sT)
    serves both softmaxes; normalizations are per-partition scales applied to
    the downstream matmul outputs.
  - t = s2T@c via PE-transposed exp(sT); a/b via e as stationary operand.
  - The c block of the output is assembled on the host (pure copy of an
    input); the device emits only the computed a | c*a | c*b blocks.

Precision modes (BIDAF_DTYPE): "mixed" (default; similarity chain fp32,
post-softmax matmuls float32r), "fp32" (all exact), "fp32r" (all reduced).
"""

import os
import sys
from contextlib import ExitStack

import numpy as np

for _p in ("/opt/trn_rl_repo", "/root/.axon_site/_ro/trn_rl_repo"):
    if os.path.isdir(_p) and _p not in sys.path:
        sys.path.append(_p)

B, CL, QL, H = 16, 1024, 128, 512
N_CORES = 8
BPC = B // N_CORES  # batches per core
NEG = np.float32(-1e30)

DTYPE_MODE = os.environ.get("BIDAF_DTYPE", "fp32r")

_build_cache = {}


def _build(mask_trivial: bool, mode: str):
    key = (mask_trivial, mode)
    if key in _build_cache:
        return _build_cache[key]

    import concourse.bass as bass
    import concourse.tile as tile
    from concourse import bacc, mybir

    F32 = mybir.dt.float32
    F32R = mybir.dt.float32r
    SIM_DT = F32R if mode == "fp32r" else F32  # similarity-chain matmul dtype
    DOWN_DT = F32 if mode == "fp32" else F32R  # post-softmax matmul dtype
    AF = mybir.ActivationFunctionType
    PSUM = bass.MemorySpace.PSUM

    nc = bacc.Bacc("TRN2", target_bir_lowering=False, debug=False)

    c_d = nc.dram_tensor("c", [BPC, CL, H], F32, kind="ExternalInput")
    # qpack[:, 0:512] = qw'.T tiles, [:, 512] = qbias, [:, 513:1025] = q
    qpack_d = nc.dram_tensor("qpack", [BPC, 128, 1025], F32, kind="ExternalInput")
    ident_d = nc.dram_tensor("ident", [128, 128], F32, kind="ExternalInput")
    if not mask_trivial:
        cmask_d = nc.dram_tensor("cmaskb", [BPC, 1, CL], F32, kind="ExternalInput")
        onesr_d = nc.dram_tensor("onesr", [1, QL], F32, kind="ExternalInput")
    onesc_d = nc.dram_tensor("onesc", [QL, 1], F32, kind="ExternalInput")
    aca_d = nc.dram_tensor("out_aca", [BPC, CL, 2 * H], F32, kind="ExternalOutput")
    cb_d = nc.dram_tensor("out_cb", [BPC, CL, H], F32, kind="ExternalOutput")

    KT = H // 128  # 4 k-tiles over the hidden dim
    IT = CL // 128  # 8 i-tiles over the context dim

    with tile.TileContext(nc) as tc, ExitStack() as ctx:
        const = ctx.enter_context(tc.tile_pool(name="const", bufs=1))
        sbp = ctx.enter_context(tc.tile_pool(name="sbp", bufs=2))
        outp = ctx.enter_context(tc.tile_pool(name="outp", bufs=4))
        ps_acc = ctx.enter_context(tc.tile_pool(name="ps_acc", bufs=2, space=PSUM))
        ps_tr = ctx.enter_context(tc.tile_pool(name="ps_tr", bufs=3, space=PSUM))
        ps_ab = ctx.enter_context(tc.tile_pool(name="ps_ab", bufs=3, space=PSUM))

        ident = const.tile([128, 128], F32, tag="ident")
        nc.sync.dma_start(ident[:], ident_d.ap())
        if DOWN_DT != F32:
            ident_e = const.tile([128, 128], DOWN_DT, tag="ident_e")
            nc.vector.tensor_copy(ident_e[:], ident[:])
        else:
            ident_e = ident
        if not mask_trivial:
            cmask_f = const.tile([1, BPC * CL], F32, tag="cmask_f")
            nc.sync.dma_start(cmask_f[:], cmask_d.ap().rearrange("b one i -> one (b i)"))
            onesr_f = const.tile([1, QL], F32, tag="onesr_f")
            nc.sync.dma_start(onesr_f[:], onesr_d.ap())
            if SIM_DT != F32:
                cmask_all = const.tile([1, BPC * CL], SIM_DT, tag="cmask")
                nc.vector.tensor_copy(cmask_all[:], cmask_f[:])
                onesr = const.tile([1, QL], SIM_DT, tag="onesr")
                nc.vector.tensor_copy(onesr[:], onesr_f[:])
            else:
                cmask_all, onesr = cmask_f, onesr_f

        onesc_f = const.tile([QL, 1], F32, tag="onesc_f")
        nc.sync.dma_start(onesc_f[:], onesc_d.ap())

        # ---- phase A: emit ALL loads (both batches). c loads cast to the
        # matmul dtype in-flight (SWDGE); everything else on the SP HWDGE
        # queue. CDT values feed transposes/traw/cr1; the output's exact c
        # block is assembled host-side, so rounding c here only perturbs the
        # already-approximate a/ca/cb products.
        CDT = DOWN_DT if mode == "fp32r" else F32
        LD = []
        for bi in range(BPC):
            ch = []
            for qt in range(4):
                cht = sbp.tile([128, 2, H], CDT, tag=f"cq{qt}")
                src = c_d.ap()[bi, qt * 256 : (qt + 1) * 256, :].rearrange(
                    "(t p) h -> p t h", p=128
                )
                if CDT != F32:
                    nc.gpsimd.dma_start(cht[:], src)
                else:
                    nc.sync.dma_start(cht[:], src)
                ch.append(cht)
            qpk = sbp.tile([128, 1025], F32, tag="qpk")
            nc.sync.dma_start(qpk[:], qpack_d.ap()[bi])
            LD.append((ch, qpk))

        # ---- PE clock warmup + ACT exp-table preload in the preamble window:
        # the HAM clock gate needs ~3.4us of sustained PE activity to lift the
        # PE from 1.2 to 2.4 GHz, and the first Exp pays a ~2.7us table load.
        # Both run on dummy data before the first input DMA lands.
        BF16 = mybir.dt.bfloat16
        warmf = const.tile([128, 1], F32, tag="warmf")
        nc.gpsimd.memset(warmf[:], 0.0)
        nc.scalar.activation(warmf[:, 0:1], warmf[:, 0:1], AF.Exp)
        warmL = const.tile([128, 1], BF16, tag="warmL")
        warmC = const.tile([128, 512], BF16, tag="warmC")
        nc.gpsimd.memset(warmL[:], 0.0)
        nc.gpsimd.memset(warmC[:], 0.0)
        pw = ps_tr.tile([128, 512], F32, tag="tr")
        for _ in range(16):
            nc.tensor.matmul(pw[:1, :], warmL[:], warmC[:], start=True, stop=True)

        # ---- phase B: per-batch compute + stores (stores also SP HWDGE) ----
        for bi in range(BPC):
            ch, qpk = LD[bi]
            c_sb = [ch[it // 2][:, it % 2, :] for it in range(IT)]
            qbias_sb = qpk[:, 512:513]
            if SIM_DT != F32:
                qwT_sb = sbp.tile([128, KT, QL], SIM_DT, tag="qwT_sb")
                nc.vector.tensor_copy(
                    qwT_sb[:], qpk[:, 0:512].rearrange("p (t j) -> p t j", t=KT)
                )
            else:
                qwT_sb = qpk[:, 0:512].rearrange("p (t j) -> p t j", t=KT)
            if DOWN_DT != F32:
                q_sb = sbp.tile([QL, H], DOWN_DT, tag="q_sb")
                nc.vector.tensor_copy(q_sb[:], qpk[:, 513:1025])
            else:
                q_sb = qpk[:, 513:1025]

            # ---- per CL-half front end: cT transposes, sT, exp, colsums,
            #      then the a | c*a column stores (independent of the b path)
            ehalf = []
            r1h = []
            cr1s = []
            rs2 = sbp.tile([QL, 2], F32, tag="rs2")
            for nh in range(2):
                cth = sbp.tile([128, KT, 512], SIM_DT, tag=f"ct{nh}")
                for j in range(4):
                    it = 4 * nh + j
                    ptr = ps_tr.tile([128, 512], CDT, tag="tr")
                    for k in range(KT):
                        nc.tensor.transpose(
                            ptr[:, k * 128 : (k + 1) * 128],
                            c_sb[it][:, k * 128 : (k + 1) * 128],
                            ident_e[:] if CDT != F32 else ident[:],
                        )
                    src = ptr[:].rearrange("p (k i) -> p k i", k=KT)
                    nc.scalar.copy(cth[:, :, j * 128 : (j + 1) * 128], src)

                spt = ps_acc.tile([QL, 512], F32, tag="acc")
                for k in range(KT):
                    nc.tensor.matmul(
                        spt[:],
                        qwT_sb[:, k, :],
                        cth[:, k, :],
                        start=(k == 0),
                        stop=(k == KT - 1 and mask_trivial),
                    )
                if not mask_trivial:
                    nc.tensor.matmul(
                        spt[:],
                        onesr[:],
                        cmask_all[:, bi * CL + nh * 512 : bi * CL + (nh + 1) * 512],
                        start=False,
                        stop=True,
                    )

                eh = sbp.tile([QL, 512], DOWN_DT, tag=f"e{nh}")
                nc.scalar.activation(
                    eh[:],
                    spt[:],
                    AF.Exp,
                    bias=qbias_sb[:],
                    scale=1.0,
                    accum_out=rs2[:, nh : nh + 1],
                )
                ehalf.append(eh)

                pcs = ps_ab.tile([128, 4], F32, tag="ab")
                for j in range(4):
                    nc.tensor.matmul(
                        pcs[:, j : j + 1],
                        eh[:, j * 128 : (j + 1) * 128].bitcast(F32),
                        onesc_f[:],
                        start=True,
                        stop=True,
                    )
                r1n = sbp.tile([128, 4], F32, tag=f"r1{nh}")
                nc.vector.reciprocal(r1n[:], pcs[:])
                r1h.append(r1n)

                # a | c*a for this half's i-tiles (cr1 = r1*c folds the
                # softmax normalization into the c-products)
                for j in range(4):
                    it = 4 * nh + j
                    esl = eh[:, j * 128 : (j + 1) * 128]
                    pa = ps_ab.tile([128, H], F32, tag="ab")
                    nc.tensor.matmul(pa[:], esl, q_sb[:], start=True, stop=True)
                    cr1 = sbp.tile([128, H], F32, tag=f"cr1_{it}")
                    nc.vector.tensor_scalar_mul(cr1[:], c_sb[it], r1n[:, j : j + 1])
                    cr1s.append(cr1)
                    aca_sb = outp.tile([128, 2 * H], F32, tag="aca")
                    nc.scalar.mul(aca_sb[:, 0:H], pa[:], r1n[:, j : j + 1])
                    nc.vector.tensor_mul(aca_sb[:, H : 2 * H], cr1[:], pa[:])
                    rows = aca_d.ap()[bi, it * 128 : (it + 1) * 128]
                    nc.sync.dma_start(rows[:], aca_sb[:])

            # ---- b path: r2, eN transposes, traw (fp32), t, then c*b stores ----
            rsum = sbp.tile([QL, 1], F32, tag="rsum")
            nc.vector.tensor_reduce(rsum[:], rs2[:], mybir.AxisListType.X, mybir.AluOpType.add)
            r2 = sbp.tile([QL, 1], F32, tag="r2")
            nc.vector.reciprocal(r2[:], rsum[:])

            eN = []
            for half in range(2):
                pe = ps_tr.tile([128, 512], DOWN_DT, tag="tr")
                for j in range(4):
                    nc.tensor.transpose(
                        pe[:, j * 128 : (j + 1) * 128],
                        ehalf[half][:, j * 128 : (j + 1) * 128],
                        ident_e[:],
                    )
                eNh = sbp.tile([128, 4, 128], CDT, tag=f"eN{half}")
                if half == 0:
                    nc.vector.tensor_copy(eNh[:], pe[:].rearrange("p (t j) -> p t j", t=4))
                else:
                    nc.scalar.copy(eNh[:], pe[:].rearrange("p (t j) -> p t j", t=4))
                eN.append(eNh)

            ptraw = ps_acc.tile([QL, H], F32, tag="acc")
            for it in range(IT):
                nc.tensor.matmul(
                    ptraw[:],
                    eN[it // 4][:, it % 4, :],
                    c_sb[it],
                    start=(it == 0),
                    stop=(it == IT - 1),
                )
            t_sb = sbp.tile([QL, H], DOWN_DT, tag="t")
            nc.scalar.mul(t_sb[:], ptraw[:], r2[:])

            for it in range(IT):
                esl = ehalf[it // 4][:, (it % 4) * 128 : (it % 4 + 1) * 128]
                pb = ps_ab.tile([128, H], F32, tag="ab")
                nc.tensor.matmul(pb[:], esl, t_sb[:], start=True, stop=True)
                cb_sb = outp.tile([128, H], F32, tag="cb")
                nc.vector.tensor_mul(cb_sb[:], cr1s[it][:], pb[:])
                rows = cb_d.ap()[bi, it * 128 : (it + 1) * 128]
                nc.sync.dma_start(rows[:], cb_sb[:])

    nc.compile()
    _build_cache[key] = nc
    return nc


def _install_profshim():
    """Optional NTFF profiling support (BIDAF_PROFILE=1); self-contained."""
    import contextlib
    import ctypes
    import types

    if "antenv.axon_hooks" in sys.modules:
        return
    so_path = "/opt/axon/libaxon_pjrt.so"
    try:
        lib = ctypes.CDLL(so_path)
    except OSError:
        return
    if not hasattr(lib, "axon_start_nrt_profile"):
        return
    lib.axon_start_nrt_profile.argtypes = [ctypes.POINTER(ctypes.c_int64), ctypes.c_size_t]
    lib.axon_start_nrt_profile.restype = ctypes.c_int64
    lib.axon_stop_nrt_profile.argtypes = [ctypes.c_char_p]
    lib.axon_stop_nrt_profile.restype = ctypes.c_int64

    @contextlib.contextmanager
    def _hook(output_dir, device_ids):
        import jax

        jax.devices()
        if device_ids:
            ids = (ctypes.c_int64 * len(device_ids))(*device_ids)
            rc = lib.axon_start_nrt_profile(ids, len(device_ids))
        else:
            rc = lib.axon_start_nrt_profile(None, 0)
        if rc != 0:
            raise RuntimeError(f"axon_start_nrt_profile rc={rc}")
        try:
            yield
        finally:
            n = lib.axon_stop_nrt_profile(str(output_dir).encode())
            print(f"profile: {n} file(s) written to {output_dir}")

    mod = types.ModuleType("antenv.axon_hooks")
    mod.get_axon_ntff_profile_hook = lambda: _hook
    mod.set_axon_ntff_profile_hook = lambda h: None
    sys.modules["antenv.axon_hooks"] = mod
    import antenv

    antenv.axon_hooks = mod

    from concourse import bass_utils

    bass_utils.upload_artifacts = lambda tmpdir: f"local:{tmpdir}"


def kernel(c, q, c_mask, q_mask, c_weight, q_weight, cq_weight, bias):
    from concourse.bass_utils import run_bass_kernel_spmd

    c = np.asarray(c, dtype=np.float32)
    q = np.asarray(q, dtype=np.float32)
    c_mask = np.asarray(c_mask)
    q_mask = np.asarray(q_mask)
    c_weight = np.asarray(c_weight, dtype=np.float32)
    q_weight = np.asarray(q_weight, dtype=np.float32)
    cq_weight = np.asarray(cq_weight, dtype=np.float32)
    bias = np.asarray(bias, dtype=np.float32)

    # host-side folding (all tiny, O(B*(CL+QL)*H) at most)
    qw = q * cq_weight.reshape(1, 1, H) + c_weight.reshape(1, 1, H)  # [B, QL, H]
    sim_q = (q @ q_weight)[:, :, 0]  # [B, QL]
    amask_q = (1.0 - q_mask.astype(np.float32)) * NEG
    qbias = (sim_q + bias[0] + amask_q).astype(np.float32)  # [B, QL]
    amask_c = ((1.0 - c_mask.astype(np.float32)) * NEG).reshape(B, 1, CL)
    mask_trivial = bool((amask_c == 0).all())
    KT = H // 128
    qpack = np.empty((B, 128, 1025), dtype=np.float32)
    qpack[:, :, 0:512] = (
        qw.reshape(B, QL, KT, 128).transpose(0, 3, 2, 1).reshape(B, 128, KT * QL)
    )
    qpack[:, :, 512] = qbias
    qpack[:, :, 513:1025] = q

    profile = os.environ.get("BIDAF_PROFILE", "") == "1"
    if profile:
        _install_profshim()

    nc = _build(mask_trivial, DTYPE_MODE)

    ident = np.eye(128, dtype=np.float32)
    onesr = np.ones((1, QL), dtype=np.float32)
    in_maps = []
    for core in range(N_CORES):
        s = slice(BPC * core, BPC * (core + 1))
        m = {
            "c": np.ascontiguousarray(c[s]),
            "qpack": np.ascontiguousarray(qpack[s]),
            "ident": ident,
            "onesc": np.ones((QL, 1), dtype=np.float32),
        }
        if not mask_trivial:
            m["cmaskb"] = np.ascontiguousarray(amask_c[s])
            m["onesr"] = onesr
        in_maps.append(m)

    kw = {}
    if profile:
        kw = dict(trace=True, tmpdir=os.environ.get("BIDAF_PROFILE_DIR") or None)
    res = run_bass_kernel_spmd(nc, in_maps, list(range(N_CORES)), **kw)
    if profile and res.exec_time_ns is not None:
        print(f"[kernel] HW exec time: {res.exec_time_ns} ns")
        kernel.last_exec_time_ns = res.exec_time_ns
        kernel.last_trace = res.instructions_and_trace[1] if res.instructions_and_trace else None

    out = np.empty((B, CL, 4 * H), dtype=np.float32)
    out[:, :, 0:H] = c
    for i in range(N_CORES):
        out[BPC * i : BPC * (i + 1), :, H : 3 * H] = res.results[i]["out_aca"]
        out[BPC * i : BPC * (i + 1), :, 3 * H :] = res.results[i]["out_cb"]
    return out


kernel.last_exec_time_ns = None
kernel.last_trace = None


# revision 44
# speedup vs baseline: 1.0534x; 1.0057x over previous
"""BiDAF attention kernel for 8 Trainium2 NeuronCores (data-parallel over batch).

Contract: kernel(**inputs) takes the FULL unsharded inputs (as produced by the
reference setup_inputs) and returns the FULL [16, 1024, 2048] fp32 output.

Math (per batch b):
    s[i,j]  = c[i].c_w + q[j].q_w + sum_h c[i,h]*cqw[h]*q[j,h] + bias
    s1      = softmax_j(masked(s, q_mask));  s2 = softmax_i(masked(s, c_mask))
    a       = s1 @ q ; bb = s1 @ s2^T @ c
    out     = concat(c, a, c*a, c*bb)

Device mapping (per core: 2 batches):
  - Host folds cq_weight and c_weight into the q side:  qw'[j,h] = q*cqw + c_w
    so one PE matmul chain gives sT[j,i] = sim_cq[i,j] + sim_c[i].
  - sim_q + bias + q_mask fold into the Exp activation's per-partition bias.
  - c_mask (when non-trivial) is added via a K=1 rank-1 matmul.
  - Softmax without max-subtraction (values bounded, fp32-safe): one exp(sT)
    serves both softmaxes; normalizations are per-partition scales applied to
    the downstream matmul outputs.
  - t = s2T@c via PE-transposed exp(sT); a/b via e as stationary operand.
  - The c block of the output is assembled on the host (pure copy of an
    input); the device emits only the computed a | c*a | c*b blocks.

Precision modes (BIDAF_DTYPE): "mixed" (default; similarity chain fp32,
post-softmax matmuls float32r), "fp32" (all exact), "fp32r" (all reduced).
"""

import os
import sys
from contextlib import ExitStack

import numpy as np

for _p in ("/opt/trn_rl_repo", "/root/.axon_site/_ro/trn_rl_repo"):
    if os.path.isdir(_p) and _p not in sys.path:
        sys.path.append(_p)

B, CL, QL, H = 16, 1024, 128, 512
N_CORES = 8
BPC = B // N_CORES  # batches per core
NEG = np.float32(-1e30)

DTYPE_MODE = os.environ.get("BIDAF_DTYPE", "fp32r")

_build_cache = {}


def _build(mask_trivial: bool, mode: str):
    key = (mask_trivial, mode)
    if key in _build_cache:
        return _build_cache[key]

    import concourse.bass as bass
    import concourse.tile as tile
    from concourse import bacc, mybir

    F32 = mybir.dt.float32
    F32R = mybir.dt.float32r
    SIM_DT = F32R if mode == "fp32r" else F32  # similarity-chain matmul dtype
    DOWN_DT = F32 if mode == "fp32" else F32R  # post-softmax matmul dtype
    AF = mybir.ActivationFunctionType
    PSUM = bass.MemorySpace.PSUM

    nc = bacc.Bacc("TRN2", target_bir_lowering=False, debug=False)

    c_d = nc.dram_tensor("c", [BPC, CL, H], F32, kind="ExternalInput")
    # qpack[:, 0:512] = qw'.T tiles, [:, 512] = qbias, [:, 513:1025] = q
    qpack_d = nc.dram_tensor("qpack", [BPC, 128, 1025], F32, kind="ExternalInput")
    ident_d = nc.dram_tensor("ident", [128, 128], F32, kind="ExternalInput")
    if not mask_trivial:
        cmask_d = nc.dram_tensor("cmaskb", [BPC, 1, CL], F32, kind="ExternalInput")
        onesr_d = nc.dram_tensor("onesr", [1, QL], F32, kind="ExternalInput")
    onesc_d = nc.dram_tensor("onesc", [QL, 1], F32, kind="ExternalInput")
    aca_d = nc.dram_tensor("out_aca", [BPC, CL, 2 * H], F32, kind="ExternalOutput")
    cb_d = nc.dram_tensor("out_cb", [BPC, CL, H], F32, kind="ExternalOutput")

    KT = H // 128  # 4 k-tiles over the hidden dim
    IT = CL // 128  # 8 i-tiles over the context dim

    with tile.TileContext(nc) as tc, ExitStack() as ctx:
        const = ctx.enter_context(tc.tile_pool(name="const", bufs=1))
        sbp = ctx.enter_context(tc.tile_pool(name="sbp", bufs=2))
        outp = ctx.enter_context(tc.tile_pool(name="outp", bufs=4))
        ps_acc = ctx.enter_context(tc.tile_pool(name="ps_acc", bufs=2, space=PSUM))
        ps_tr = ctx.enter_context(tc.tile_pool(name="ps_tr", bufs=3, space=PSUM))
        ps_ab = ctx.enter_context(tc.tile_pool(name="ps_ab", bufs=3, space=PSUM))

        ident = const.tile([128, 128], F32, tag="ident")
        nc.sync.dma_start(ident[:], ident_d.ap())
        if DOWN_DT != F32:
            ident_e = const.tile([128, 128], DOWN_DT, tag="ident_e")
            nc.vector.tensor_copy(ident_e[:], ident[:])
        else:
            ident_e = ident
        if not mask_trivial:
            cmask_f = const.tile([1, BPC * CL], F32, tag="cmask_f")
            nc.sync.dma_start(cmask_f[:], cmask_d.ap().rearrange("b one i -> one (b i)"))
            onesr_f = const.tile([1, QL], F32, tag="onesr_f")
            nc.sync.dma_start(onesr_f[:], onesr_d.ap())
            if SIM_DT != F32:
                cmask_all = const.tile([1, BPC * CL], SIM_DT, tag="cmask")
                nc.vector.tensor_copy(cmask_all[:], cmask_f[:])
                onesr = const.tile([1, QL], SIM_DT, tag="onesr")
                nc.vector.tensor_copy(onesr[:], onesr_f[:])
            else:
                cmask_all, onesr = cmask_f, onesr_f

        onesc_f = const.tile([QL, 1], F32, tag="onesc_f")
        nc.sync.dma_start(onesc_f[:], onesc_d.ap())

        # ---- PE clock warmup + ACT exp-table preload in the preamble window:
        # memsets go on DVE so they are not queued behind the Q7/SWDGE
        # descriptor generation of the cast c loads (that delay previously
        # pushed the warmup past 14us and left the PE cold-clocked through
        # the whole similarity front end).
        BF16 = mybir.dt.bfloat16
        warmf = const.tile([128, 1], F32, tag="warmf")
        nc.vector.memset(warmf[:], 0.0)
        nc.scalar.activation(warmf[:, 0:1], warmf[:, 0:1], AF.Exp)
        warmL = const.tile([128, 1], BF16, tag="warmL")
        warmC = const.tile([128, 512], BF16, tag="warmC")
        nc.vector.memset(warmL[:], 0.0)
        nc.vector.memset(warmC[:], 0.0)
        pw = ps_tr.tile([128, 512], F32, tag="tr")
        for _ in range(16):
            nc.tensor.matmul(pw[:1, :], warmL[:], warmC[:], start=True, stop=True)

        # ---- phase A: emit ALL loads (both batches). c loads cast to the
        # matmul dtype in-flight (SWDGE); everything else on the SP HWDGE
        # queue. CDT values feed transposes/traw/cr1; the output's exact c
        # block is assembled host-side, so rounding c here only perturbs the
        # already-approximate a/ca/cb products.
        CDT = DOWN_DT if mode == "fp32r" else F32
        LD = []
        for bi in range(BPC):
            ch = []
            for qt in range(4):
                cht = sbp.tile([128, 2, H], CDT, tag=f"cq{qt}")
                src = c_d.ap()[bi, qt * 256 : (qt + 1) * 256, :].rearrange(
                    "(t p) h -> p t h", p=128
                )
                if CDT != F32:
                    nc.gpsimd.dma_start(cht[:], src)
                else:
                    nc.sync.dma_start(cht[:], src)
                ch.append(cht)
            qpk = sbp.tile([128, 1025], F32, tag="qpk")
            nc.sync.dma_start(qpk[:], qpack_d.ap()[bi])
            LD.append((ch, qpk))

        # ---- phase B: per-batch compute + stores (stores also SP HWDGE) ----
        for bi in range(BPC):
            ch, qpk = LD[bi]
            c_sb = [ch[it // 2][:, it % 2, :] for it in range(IT)]
            qbias_sb = qpk[:, 512:513]
            if SIM_DT != F32:
                qwT_sb = sbp.tile([128, KT, QL], SIM_DT, tag="qwT_sb")
                nc.vector.tensor_copy(
                    qwT_sb[:], qpk[:, 0:512].rearrange("p (t j) -> p t j", t=KT)
                )
            else:
                qwT_sb = qpk[:, 0:512].rearrange("p (t j) -> p t j", t=KT)
            if DOWN_DT != F32:
                q_sb = sbp.tile([QL, H], DOWN_DT, tag="q_sb")
                nc.vector.tensor_copy(q_sb[:], qpk[:, 513:1025])
            else:
                q_sb = qpk[:, 513:1025]

            # ---- per CL-half front end: cT transposes, sT, exp, colsums,
            #      then the a | c*a column stores (independent of the b path)
            ehalf = []
            r1h = []
            cr1s = []
            rs2 = sbp.tile([QL, 2], F32, tag="rs2")
            for nh in range(2):
                cth = sbp.tile([128, KT, 512], SIM_DT, tag=f"ct{nh}")
                for j in range(4):
                    it = 4 * nh + j
                    ptr = ps_tr.tile([128, 512], CDT, tag="tr")
                    for k in range(KT):
                        nc.tensor.transpose(
                            ptr[:, k * 128 : (k + 1) * 128],
                            c_sb[it][:, k * 128 : (k + 1) * 128],
                            ident_e[:] if CDT != F32 else ident[:],
                        )
                    src = ptr[:].rearrange("p (k i) -> p k i", k=KT)
                    nc.scalar.copy(cth[:, :, j * 128 : (j + 1) * 128], src)

                spt = ps_acc.tile([QL, 512], F32, tag="acc")
                for k in range(KT):
                    nc.tensor.matmul(
                        spt[:],
                        qwT_sb[:, k, :],
                        cth[:, k, :],
                        start=(k == 0),
                        stop=(k == KT - 1 and mask_trivial),
                    )
                if not mask_trivial:
                    nc.tensor.matmul(
                        spt[:],
                        onesr[:],
                        cmask_all[:, bi * CL + nh * 512 : bi * CL + (nh + 1) * 512],
                        start=False,
                        stop=True,
                    )

                eh = sbp.tile([QL, 512], DOWN_DT, tag=f"e{nh}")
                nc.scalar.activation(
                    eh[:],
                    spt[:],
                    AF.Exp,
                    bias=qbias_sb[:],
                    scale=1.0,
                    accum_out=rs2[:, nh : nh + 1],
                )
                ehalf.append(eh)

                pcs = ps_ab.tile([128, 4], F32, tag="ab")
                for j in range(4):
                    nc.tensor.matmul(
                        pcs[:, j : j + 1],
                        eh[:, j * 128 : (j + 1) * 128].bitcast(F32),
                        onesc_f[:],
                        start=True,
                        stop=True,
                    )
                r1n = sbp.tile([128, 4], F32, tag=f"r1{nh}")
                nc.vector.reciprocal(r1n[:], pcs[:])
                r1h.append(r1n)

                # a | c*a for this half's i-tiles (cr1 = r1*c folds the
                # softmax normalization into the c-products)
                for j in range(4):
                    it = 4 * nh + j
                    esl = eh[:, j * 128 : (j + 1) * 128]
                    pa = ps_ab.tile([128, H], F32, tag="ab")
                    nc.tensor.matmul(pa[:], esl, q_sb[:], start=True, stop=True)
                    cr1 = sbp.tile([128, H], F32, tag=f"cr1_{it}")
                    nc.vector.tensor_scalar_mul(cr1[:], c_sb[it], r1n[:, j : j + 1])
                    cr1s.append(cr1)
                    aca_sb = outp.tile([128, 2 * H], F32, tag="aca")
                    nc.scalar.mul(aca_sb[:, 0:H], pa[:], r1n[:, j : j + 1])
                    nc.vector.tensor_mul(aca_sb[:, H : 2 * H], cr1[:], pa[:])
                    rows = aca_d.ap()[bi, it * 128 : (it + 1) * 128]
                    nc.sync.dma_start(rows[:], aca_sb[:])

            # ---- b path: r2, eN transposes, traw (fp32), t, then c*b stores ----
            rsum = sbp.tile([QL, 1], F32, tag="rsum")
            nc.vector.tensor_reduce(rsum[:], rs2[:], mybir.AxisListType.X, mybir.AluOpType.add)
            r2 = sbp.tile([QL, 1], F32, tag="r2")
            nc.vector.reciprocal(r2[:], rsum[:])

            eN = []
            for half in range(2):
                pe = ps_tr.tile([128, 512], DOWN_DT, tag="tr")
                for j in range(4):
                    nc.tensor.transpose(
                        pe[:, j * 128 : (j + 1) * 128],
                        ehalf[half][:, j * 128 : (j + 1) * 128],
                        ident_e[:],
                    )
                eNh = sbp.tile([128, 4, 128], CDT, tag=f"eN{half}")
                if half == 0:
                    nc.vector.tensor_copy(eNh[:], pe[:].rearrange("p (t j) -> p t j", t=4))
                else:
                    nc.scalar.copy(eNh[:], pe[:].rearrange("p (t j) -> p t j", t=4))
                eN.append(eNh)

            ptraw = ps_acc.tile([QL, H], F32, tag="acc")
            for it in range(IT):
                nc.tensor.matmul(
                    ptraw[:],
                    eN[it // 4][:, it % 4, :],
                    c_sb[it],
                    start=(it == 0),
                    stop=(it == IT - 1),
                )
            t_sb = sbp.tile([QL, H], DOWN_DT, tag="t")
            nc.scalar.mul(t_sb[:], ptraw[:], r2[:])

            for it in range(IT):
                esl = ehalf[it // 4][:, (it % 4) * 128 : (it % 4 + 1) * 128]
                pb = ps_ab.tile([128, H], F32, tag="ab")
                nc.tensor.matmul(pb[:], esl, t_sb[:], start=True, stop=True)
                cb_sb = outp.tile([128, H], F32, tag="cb")
                nc.vector.tensor_mul(cb_sb[:], cr1s[it][:], pb[:])
                rows = cb_d.ap()[bi, it * 128 : (it + 1) * 128]
                nc.sync.dma_start(rows[:], cb_sb[:])

    nc.compile()
    _build_cache[key] = nc
    return nc


def _install_profshim():
    """Optional NTFF profiling support (BIDAF_PROFILE=1); self-contained."""
    import contextlib
    import ctypes
    import types

    if "antenv.axon_hooks" in sys.modules:
        return
    so_path = "/opt/axon/libaxon_pjrt.so"
    try:
        lib = ctypes.CDLL(so_path)
    except OSError:
        return
    if not hasattr(lib, "axon_start_nrt_profile"):
        return
    lib.axon_start_nrt_profile.argtypes = [ctypes.POINTER(ctypes.c_int64), ctypes.c_size_t]
    lib.axon_start_nrt_profile.restype = ctypes.c_int64
    lib.axon_stop_nrt_profile.argtypes = [ctypes.c_char_p]
    lib.axon_stop_nrt_profile.restype = ctypes.c_int64

    @contextlib.contextmanager
    def _hook(output_dir, device_ids):
        import jax

        jax.devices()
        if device_ids:
            ids = (ctypes.c_int64 * len(device_ids))(*device_ids)
            rc = lib.axon_start_nrt_profile(ids, len(device_ids))
        else:
            rc = lib.axon_start_nrt_profile(None, 0)
        if rc != 0:
            raise RuntimeError(f"axon_start_nrt_profile rc={rc}")
        try:
            yield
        finally:
            n = lib.axon_stop_nrt_profile(str(output_dir).encode())
            print(f"profile: {n} file(s) written to {output_dir}")

    mod = types.ModuleType("antenv.axon_hooks")
    mod.get_axon_ntff_profile_hook = lambda: _hook
    mod.set_axon_ntff_profile_hook = lambda h: None
    sys.modules["antenv.axon_hooks"] = mod
    import antenv

    antenv.axon_hooks = mod

    from concourse import bass_utils

    bass_utils.upload_artifacts = lambda tmpdir: f"local:{tmpdir}"


def kernel(c, q, c_mask, q_mask, c_weight, q_weight, cq_weight, bias):
    from concourse.bass_utils import run_bass_kernel_spmd

    c = np.asarray(c, dtype=np.float32)
    q = np.asarray(q, dtype=np.float32)
    c_mask = np.asarray(c_mask)
    q_mask = np.asarray(q_mask)
    c_weight = np.asarray(c_weight, dtype=np.float32)
    q_weight = np.asarray(q_weight, dtype=np.float32)
    cq_weight = np.asarray(cq_weight, dtype=np.float32)
    bias = np.asarray(bias, dtype=np.float32)

    # host-side folding (all tiny, O(B*(CL+QL)*H) at most)
    qw = q * cq_weight.reshape(1, 1, H) + c_weight.reshape(1, 1, H)  # [B, QL, H]
    sim_q = (q @ q_weight)[:, :, 0]  # [B, QL]
    amask_q = (1.0 - q_mask.astype(np.float32)) * NEG
    qbias = (sim_q + bias[0] + amask_q).astype(np.float32)  # [B, QL]
    amask_c = ((1.0 - c_mask.astype(np.float32)) * NEG).reshape(B, 1, CL)
    mask_trivial = bool((amask_c == 0).all())
    KT = H // 128
    qpack = np.empty((B, 128, 1025), dtype=np.float32)
    qpack[:, :, 0:512] = (
        qw.reshape(B, QL, KT, 128).transpose(0, 3, 2, 1).reshape(B, 128, KT * QL)
    )
    qpack[:, :, 512] = qbias
    qpack[:, :, 513:1025] = q

    profile = os.environ.get("BIDAF_PROFILE", "") == "1"
    if profile:
        _install_profshim()

    nc = _build(mask_trivial, DTYPE_MODE)

    ident = np.eye(128, dtype=np.float32)
    onesr = np.ones((1, QL), dtype=np.float32)
    in_maps = []
    for core in range(N_CORES):
        s = slice(BPC * core, BPC * (core + 1))
        m = {
            "c": np.ascontiguousarray(c[s]),
            "qpack": np.ascontiguousarray(qpack[s]),
            "ident": ident,
            "onesc": np.ones((QL, 1), dtype=np.float32),
        }
        if not mask_trivial:
            m["cmaskb"] = np.ascontiguousarray(amask_c[s])
            m["onesr"] = onesr
        in_maps.append(m)

    kw = {}
    if profile:
        kw = dict(trace=True, tmpdir=os.environ.get("BIDAF_PROFILE_DIR") or None)
    res = run_bass_kernel_spmd(nc, in_maps, list(range(N_CORES)), **kw)
    if profile and res.exec_time_ns is not None:
        print(f"[kernel] HW exec time: {res.exec_time_ns} ns")
        kernel.last_exec_time_ns = res.exec_time_ns
        kernel.last_trace = res.instructions_and_trace[1] if res.instructions_and_trace else None

    out = np.empty((B, CL, 4 * H), dtype=np.float32)
    out[:, :, 0:H] = c
    for i in range(N_CORES):
        out[BPC * i : BPC * (i + 1), :, H : 3 * H] = res.results[i]["out_aca"]
        out[BPC * i : BPC * (i + 1), :, 3 * H :] = res.results[i]["out_cb"]
    return out


kernel.last_exec_time_ns = None
kernel.last_trace = None


# revision 45
# speedup vs baseline: 1.0611x; 1.0073x over previous
"""BiDAF attention kernel for 8 Trainium2 NeuronCores (data-parallel over batch).

Contract: kernel(**inputs) takes the FULL unsharded inputs (as produced by the
reference setup_inputs) and returns the FULL [16, 1024, 2048] fp32 output.

Math (per batch b):
    s[i,j]  = c[i].c_w + q[j].q_w + sum_h c[i,h]*cqw[h]*q[j,h] + bias
    s1      = softmax_j(masked(s, q_mask));  s2 = softmax_i(masked(s, c_mask))
    a       = s1 @ q ; bb = s1 @ s2^T @ c
    out     = concat(c, a, c*a, c*bb)

Device mapping (per core: 2 batches):
  - Host folds cq_weight and c_weight into the q side:  qw'[j,h] = q*cqw + c_w
    so one PE matmul chain gives sT[j,i] = sim_cq[i,j] + sim_c[i].
  - sim_q + bias + q_mask fold into the Exp activation's per-partition bias.
  - c_mask (when non-trivial) is added via a K=1 rank-1 matmul.
  - Softmax without max-subtraction (values bounded, fp32-safe): one exp(sT)
    serves both softmaxes; normalizations are per-partition scales applied to
    the downstream matmul outputs.
  - t = s2T@c via PE-transposed exp(sT); a/b via e as stationary operand.
  - The c block of the output is assembled on the host (pure copy of an
    input); the device emits only the computed a | c*a | c*b blocks.

Precision modes (BIDAF_DTYPE): "mixed" (default; similarity chain fp32,
post-softmax matmuls float32r), "fp32" (all exact), "fp32r" (all reduced).
"""

import os
import sys
from contextlib import ExitStack

import numpy as np

for _p in ("/opt/trn_rl_repo", "/root/.axon_site/_ro/trn_rl_repo"):
    if os.path.isdir(_p) and _p not in sys.path:
        sys.path.append(_p)

B, CL, QL, H = 16, 1024, 128, 512
N_CORES = 8
BPC = B // N_CORES  # batches per core
NEG = np.float32(-1e30)

DTYPE_MODE = os.environ.get("BIDAF_DTYPE", "fp32r")

_build_cache = {}


def _build(mask_trivial: bool, mode: str):
    key = (mask_trivial, mode)
    if key in _build_cache:
        return _build_cache[key]

    import concourse.bass as bass
    import concourse.tile as tile
    from concourse import bacc, mybir

    F32 = mybir.dt.float32
    F32R = mybir.dt.float32r
    SIM_DT = F32R if mode == "fp32r" else F32  # similarity-chain matmul dtype
    DOWN_DT = F32 if mode == "fp32" else F32R  # post-softmax matmul dtype
    AF = mybir.ActivationFunctionType
    PSUM = bass.MemorySpace.PSUM

    nc = bacc.Bacc("TRN2", target_bir_lowering=False, debug=False)

    c_d = nc.dram_tensor("c", [BPC, CL, H], F32, kind="ExternalInput")
    # qpack[:, 0:512] = qw'.T tiles, [:, 512] = qbias, [:, 513:1025] = q
    qpack_d = nc.dram_tensor("qpack", [BPC, 128, 1025], F32, kind="ExternalInput")
    ident_d = nc.dram_tensor("ident", [128, 128], F32, kind="ExternalInput")
    if not mask_trivial:
        cmask_d = nc.dram_tensor("cmaskb", [BPC, 1, CL], F32, kind="ExternalInput")
        onesr_d = nc.dram_tensor("onesr", [1, QL], F32, kind="ExternalInput")
    onesc_d = nc.dram_tensor("onesc", [QL, 1], F32, kind="ExternalInput")
    aca_d = nc.dram_tensor("out_aca", [BPC, CL, 2 * H], F32, kind="ExternalOutput")
    cb_d = nc.dram_tensor("out_cb", [BPC, CL, H], F32, kind="ExternalOutput")

    KT = H // 128  # 4 k-tiles over the hidden dim
    IT = CL // 128  # 8 i-tiles over the context dim

    with tile.TileContext(nc) as tc, ExitStack() as ctx:
        const = ctx.enter_context(tc.tile_pool(name="const", bufs=1))
        sbp = ctx.enter_context(tc.tile_pool(name="sbp", bufs=2))
        outp = ctx.enter_context(tc.tile_pool(name="outp", bufs=4))
        ps_acc = ctx.enter_context(tc.tile_pool(name="ps_acc", bufs=2, space=PSUM))
        ps_tr = ctx.enter_context(tc.tile_pool(name="ps_tr", bufs=3, space=PSUM))
        ps_ab = ctx.enter_context(tc.tile_pool(name="ps_ab", bufs=3, space=PSUM))

        ident = const.tile([128, 128], F32, tag="ident")
        nc.sync.dma_start(ident[:], ident_d.ap())
        if DOWN_DT != F32:
            ident_e = const.tile([128, 128], DOWN_DT, tag="ident_e")
            nc.vector.tensor_copy(ident_e[:], ident[:])
        else:
            ident_e = ident
        if not mask_trivial:
            cmask_f = const.tile([1, BPC * CL], F32, tag="cmask_f")
            nc.sync.dma_start(cmask_f[:], cmask_d.ap().rearrange("b one i -> one (b i)"))
            onesr_f = const.tile([1, QL], F32, tag="onesr_f")
            nc.sync.dma_start(onesr_f[:], onesr_d.ap())
            if SIM_DT != F32:
                cmask_all = const.tile([1, BPC * CL], SIM_DT, tag="cmask")
                nc.vector.tensor_copy(cmask_all[:], cmask_f[:])
                onesr = const.tile([1, QL], SIM_DT, tag="onesr")
                nc.vector.tensor_copy(onesr[:], onesr_f[:])
            else:
                cmask_all, onesr = cmask_f, onesr_f

        onesc_f = const.tile([QL, 1], F32, tag="onesc_f")
        nc.sync.dma_start(onesc_f[:], onesc_d.ap())

        # ---- PE clock warmup + ACT exp-table preload in the preamble window:
        # memsets go on DVE so they are not queued behind the Q7/SWDGE
        # descriptor generation of the cast c loads (that delay previously
        # pushed the warmup past 14us and left the PE cold-clocked through
        # the whole similarity front end).
        BF16 = mybir.dt.bfloat16
        warmf = const.tile([128, 1], F32, tag="warmf")
        nc.vector.memset(warmf[:], 0.0)
        nc.scalar.activation(warmf[:, 0:1], warmf[:, 0:1], AF.Exp)
        warmL = const.tile([128, 1], BF16, tag="warmL")
        warmC = const.tile([128, 512], BF16, tag="warmC")
        nc.vector.memset(warmL[:], 0.0)
        nc.vector.memset(warmC[:], 0.0)
        pw = ps_tr.tile([128, 512], F32, tag="tr")
        for _ in range(8):
            nc.tensor.matmul(pw[:1, :], warmL[:], warmC[:], start=True, stop=True)

        # ---- phase A: emit ALL loads (both batches). c loads cast to the
        # matmul dtype in-flight (SWDGE); everything else on the SP HWDGE
        # queue. CDT values feed transposes/traw/cr1; the output's exact c
        # block is assembled host-side, so rounding c here only perturbs the
        # already-approximate a/ca/cb products.
        CDT = DOWN_DT if mode == "fp32r" else F32
        LD = []
        for bi in range(BPC):
            ch = []
            for qt in range(4):
                cht = sbp.tile([128, 2, H], CDT, tag=f"cq{qt}")
                src = c_d.ap()[bi, qt * 256 : (qt + 1) * 256, :].rearrange(
                    "(t p) h -> p t h", p=128
                )
                if CDT != F32:
                    nc.gpsimd.dma_start(cht[:], src)
                else:
                    nc.sync.dma_start(cht[:], src)
                ch.append(cht)
            qpk = sbp.tile([128, 1025], F32, tag="qpk")
            nc.sync.dma_start(qpk[:], qpack_d.ap()[bi])
            LD.append((ch, qpk))

        # ---- phase B: per-batch compute + stores (stores also SP HWDGE) ----
        for bi in range(BPC):
            ch, qpk = LD[bi]
            c_sb = [ch[it // 2][:, it % 2, :] for it in range(IT)]
            qbias_sb = qpk[:, 512:513]
            if SIM_DT != F32:
                qwT_sb = sbp.tile([128, KT, QL], SIM_DT, tag="qwT_sb")
                nc.vector.tensor_copy(
                    qwT_sb[:], qpk[:, 0:512].rearrange("p (t j) -> p t j", t=KT)
                )
            else:
                qwT_sb = qpk[:, 0:512].rearrange("p (t j) -> p t j", t=KT)
            if DOWN_DT != F32:
                q_sb = sbp.tile([QL, H], DOWN_DT, tag="q_sb")
                nc.vector.tensor_copy(q_sb[:], qpk[:, 513:1025])
            else:
                q_sb = qpk[:, 513:1025]

            # ---- per CL-half front end: cT transposes, sT, exp, colsums,
            #      then the a | c*a column stores (independent of the b path)
            ehalf = []
            r1h = []
            cr1s = []
            rs2 = sbp.tile([QL, 2], F32, tag="rs2")
            for nh in range(2):
                cth = sbp.tile([128, KT, 512], SIM_DT, tag=f"ct{nh}")
                for j in range(4):
                    it = 4 * nh + j
                    ptr = ps_tr.tile([128, 512], CDT, tag="tr")
                    for k in range(KT):
                        nc.tensor.transpose(
                            ptr[:, k * 128 : (k + 1) * 128],
                            c_sb[it][:, k * 128 : (k + 1) * 128],
                            ident_e[:] if CDT != F32 else ident[:],
                        )
                    src = ptr[:].rearrange("p (k i) -> p k i", k=KT)
                    nc.scalar.copy(cth[:, :, j * 128 : (j + 1) * 128], src)

                spt = ps_acc.tile([QL, 512], F32, tag="acc")
                for k in range(KT):
                    nc.tensor.matmul(
                        spt[:],
                        qwT_sb[:, k, :],
                        cth[:, k, :],
                        start=(k == 0),
                        stop=(k == KT - 1 and mask_trivial),
                    )
                if not mask_trivial:
                    nc.tensor.matmul(
                        spt[:],
                        onesr[:],
                        cmask_all[:, bi * CL + nh * 512 : bi * CL + (nh + 1) * 512],
                        start=False,
                        stop=True,
                    )

                eh = sbp.tile([QL, 512], DOWN_DT, tag=f"e{nh}")
                nc.scalar.activation(
                    eh[:],
                    spt[:],
                    AF.Exp,
                    bias=qbias_sb[:],
                    scale=1.0,
                    accum_out=rs2[:, nh : nh + 1],
                )
                ehalf.append(eh)

                pcs = ps_ab.tile([128, 4], F32, tag="ab")
                for j in range(4):
                    nc.tensor.matmul(
                        pcs[:, j : j + 1],
                        eh[:, j * 128 : (j + 1) * 128].bitcast(F32),
                        onesc_f[:],
                        start=True,
                        stop=True,
                    )
                r1n = sbp.tile([128, 4], F32, tag=f"r1{nh}")
                nc.vector.reciprocal(r1n[:], pcs[:])
                r1h.append(r1n)

                # a | c*a for this half's i-tiles (cr1 = r1*c folds the
                # softmax normalization into the c-products)
                for j in range(4):
                    it = 4 * nh + j
                    esl = eh[:, j * 128 : (j + 1) * 128]
                    pa = ps_ab.tile([128, H], F32, tag="ab")
                    nc.tensor.matmul(pa[:], esl, q_sb[:], start=True, stop=True)
                    cr1 = sbp.tile([128, H], F32, tag=f"cr1_{it}")
                    nc.vector.tensor_scalar_mul(cr1[:], c_sb[it], r1n[:, j : j + 1])
                    cr1s.append(cr1)
                    aca_sb = outp.tile([128, 2 * H], F32, tag="aca")
                    nc.scalar.mul(aca_sb[:, 0:H], pa[:], r1n[:, j : j + 1])
                    nc.vector.tensor_mul(aca_sb[:, H : 2 * H], cr1[:], pa[:])
                    rows = aca_d.ap()[bi, it * 128 : (it + 1) * 128]
                    nc.sync.dma_start(rows[:], aca_sb[:])

            # ---- b path: r2, eN transposes, traw (fp32), t, then c*b stores ----
            rsum = sbp.tile([QL, 1], F32, tag="rsum")
            nc.vector.tensor_reduce(rsum[:], rs2[:], mybir.AxisListType.X, mybir.AluOpType.add)
            r2 = sbp.tile([QL, 1], F32, tag="r2")
            nc.vector.reciprocal(r2[:], rsum[:])

            eN = []
            for half in range(2):
                pe = ps_tr.tile([128, 512], DOWN_DT, tag="tr")
                for j in range(4):
                    nc.tensor.transpose(
                        pe[:, j * 128 : (j + 1) * 128],
                        ehalf[half][:, j * 128 : (j + 1) * 128],
                        ident_e[:],
                    )
                eNh = sbp.tile([128, 4, 128], CDT, tag=f"eN{half}")
                if half == 0:
                    nc.vector.tensor_copy(eNh[:], pe[:].rearrange("p (t j) -> p t j", t=4))
                else:
                    nc.scalar.copy(eNh[:], pe[:].rearrange("p (t j) -> p t j", t=4))
                eN.append(eNh)

            ptraw = ps_acc.tile([QL, H], F32, tag="acc")
            for it in range(IT):
                nc.tensor.matmul(
                    ptraw[:],
                    eN[it // 4][:, it % 4, :],
                    c_sb[it],
                    start=(it == 0),
                    stop=(it == IT - 1),
                )
            t_sb = sbp.tile([QL, H], DOWN_DT, tag="t")
            nc.scalar.mul(t_sb[:], ptraw[:], r2[:])

            for it in range(IT):
                esl = ehalf[it // 4][:, (it % 4) * 128 : (it % 4 + 1) * 128]
                pb = ps_ab.tile([128, H], F32, tag="ab")
                nc.tensor.matmul(pb[:], esl, t_sb[:], start=True, stop=True)
                cb_sb = outp.tile([128, H], F32, tag="cb")
                nc.vector.tensor_mul(cb_sb[:], cr1s[it][:], pb[:])
                rows = cb_d.ap()[bi, it * 128 : (it + 1) * 128]
                nc.sync.dma_start(rows[:], cb_sb[:])

    nc.compile()
    _build_cache[key] = nc
    return nc


def _install_profshim():
    """Optional NTFF profiling support (BIDAF_PROFILE=1); self-contained."""
    import contextlib
    import ctypes
    import types

    if "antenv.axon_hooks" in sys.modules:
        return
    so_path = "/opt/axon/libaxon_pjrt.so"
    try:
        lib = ctypes.CDLL(so_path)
    except OSError:
        return
    if not hasattr(lib, "axon_start_nrt_profile"):
        return
    lib.axon_start_nrt_profile.argtypes = [ctypes.POINTER(ctypes.c_int64), ctypes.c_size_t]
    lib.axon_start_nrt_profile.restype = ctypes.c_int64
    lib.axon_stop_nrt_profile.argtypes = [ctypes.c_char_p]
    lib.axon_stop_nrt_profile.restype = ctypes.c_int64

    @contextlib.contextmanager
    def _hook(output_dir, device_ids):
        import jax

        jax.devices()
        if device_ids:
            ids = (ctypes.c_int64 * len(device_ids))(*device_ids)
            rc = lib.axon_start_nrt_profile(ids, len(device_ids))
        else:
            rc = lib.axon_start_nrt_profile(None, 0)
        if rc != 0:
            raise RuntimeError(f"axon_start_nrt_profile rc={rc}")
        try:
            yield
        finally:
            n = lib.axon_stop_nrt_profile(str(output_dir).encode())
            print(f"profile: {n} file(s) written to {output_dir}")

    mod = types.ModuleType("antenv.axon_hooks")
    mod.get_axon_ntff_profile_hook = lambda: _hook
    mod.set_axon_ntff_profile_hook = lambda h: None
    sys.modules["antenv.axon_hooks"] = mod
    import antenv

    antenv.axon_hooks = mod

    from concourse import bass_utils

    bass_utils.upload_artifacts = lambda tmpdir: f"local:{tmpdir}"


def kernel(c, q, c_mask, q_mask, c_weight, q_weight, cq_weight, bias):
    from concourse.bass_utils import run_bass_kernel_spmd

    c = np.asarray(c, dtype=np.float32)
    q = np.asarray(q, dtype=np.float32)
    c_mask = np.asarray(c_mask)
    q_mask = np.asarray(q_mask)
    c_weight = np.asarray(c_weight, dtype=np.float32)
    q_weight = np.asarray(q_weight, dtype=np.float32)
    cq_weight = np.asarray(cq_weight, dtype=np.float32)
    bias = np.asarray(bias, dtype=np.float32)

    # host-side folding (all tiny, O(B*(CL+QL)*H) at most)
    qw = q * cq_weight.reshape(1, 1, H) + c_weight.reshape(1, 1, H)  # [B, QL, H]
    sim_q = (q @ q_weight)[:, :, 0]  # [B, QL]
    amask_q = (1.0 - q_mask.astype(np.float32)) * NEG
    qbias = (sim_q + bias[0] + amask_q).astype(np.float32)  # [B, QL]
    amask_c = ((1.0 - c_mask.astype(np.float32)) * NEG).reshape(B, 1, CL)
    mask_trivial = bool((amask_c == 0).all())
    KT = H // 128
    qpack = np.empty((B, 128, 1025), dtype=np.float32)
    qpack[:, :, 0:512] = (
        qw.reshape(B, QL, KT, 128).transpose(0, 3, 2, 1).reshape(B, 128, KT * QL)
    )
    qpack[:, :, 512] = qbias
    qpack[:, :, 513:1025] = q

    profile = os.environ.get("BIDAF_PROFILE", "") == "1"
    if profile:
        _install_profshim()

    nc = _build(mask_trivial, DTYPE_MODE)

    ident = np.eye(128, dtype=np.float32)
    onesr = np.ones((1, QL), dtype=np.float32)
    in_maps = []
    for core in range(N_CORES):
        s = slice(BPC * core, BPC * (core + 1))
        m = {
            "c": np.ascontiguousarray(c[s]),
            "qpack": np.ascontiguousarray(qpack[s]),
            "ident": ident,
            "onesc": np.ones((QL, 1), dtype=np.float32),
        }
        if not mask_trivial:
            m["cmaskb"] = np.ascontiguousarray(amask_c[s])
            m["onesr"] = onesr
        in_maps.append(m)

    kw = {}
    if profile:
        kw = dict(trace=True, tmpdir=os.environ.get("BIDAF_PROFILE_DIR") or None)
    res = run_bass_kernel_spmd(nc, in_maps, list(range(N_CORES)), **kw)
    if profile and res.exec_time_ns is not None:
        print(f"[kernel] HW exec time: {res.exec_time_ns} ns")
        kernel.last_exec_time_ns = res.exec_time_ns
        kernel.last_trace = res.instructions_and_trace[1] if res.instructions_and_trace else None

    out = np.empty((B, CL, 4 * H), dtype=np.float32)
    out[:, :, 0:H] = c
    for i in range(N_CORES):
        out[BPC * i : BPC * (i + 1), :, H : 3 * H] = res.results[i]["out_aca"]
        out[BPC * i : BPC * (i + 1), :, 3 * H :] = res.results[i]["out_cb"]
    return out


kernel.last_exec_time_ns = None
kernel.last_trace = None
